# revision 39
# baseline (speedup 1.0000x reference)
"""Trainium2 8-core kernel for a GPT-style transformer block.

Strategy:
  - Token-parallel everywhere except attention: each core owns a contiguous
    512-token span (core i -> batch i//4, span i%4). LayerNorms, QKV, proj,
    MLP and residuals are purely local to the core's tokens.
  - Attention is head-parallel: one AllToAll redistributes Q^T/K^T together
    (feature-major) and one moves V (token-major) so core j holds head-pair
    j (heads 2j, 2j+1) for ALL 4096 tokens. Every core runs the identical
    causal-attention loop (SPMD-uniform). A third AllToAll brings y back
    token-sharded.
  - Activations are kept feature-major [C, tokens] so every matmul contraction
    sits on the partition axis; LN statistics are computed with ones-vector
    matmuls on the TensorEngine and broadcast back with rank-1 matmuls.
  - Compute dtype bf16 (fp32 PSUM accumulation); residual stream fp32 in SBUF
    sourced from the bf16 x input.
  - Softmax skips max-subtraction (scores are O(1) for this problem's scale)
    and gets denominators for free via a ones-column prepended to V; the
    per-query normalization uses the fast DVE reciprocal (no scalar-engine
    ln/exp -> no activation-table thrash against the attention exps).
  - Scheduling: bulk weight streaming rides the SP DGE queue in priority
    order (xt, attn-q/k/v chunks, masks, then gated proj/fc/mlp prefetch
    that streams through the attention window when DMA is otherwise idle,
    keeping the y AllToAll uncontended). cc-buffer writes and attention
    assembly loads ride the Activation DGE queue (batched, few
    instructions); flush/output DMAs ride the GpSimd SWDGE.
"""

import sys

sys.path.insert(0, "/opt/trn_rl_repo")

import numpy as np
import ml_dtypes

import concourse.bass as bass
import concourse.mybir as mybir
import concourse.tile as tile
from concourse import bacc, bass_utils

BF16 = mybir.dt.bfloat16
FP8 = mybir.dt.float8e4
F32 = mybir.dt.float32
AF = mybir.ActivationFunctionType
ALU = mybir.AluOpType
NP_BF16 = ml_dtypes.bfloat16

B, T, C, H, HS, FF = 2, 2048, 1024, 16, 64, 4096
CORES = 8
S = 512            # tokens per core
NCT = C // 128     # 8 feature tiles
NFT = FF // 128    # 32 mlp hidden tiles
NTT = S // 128     # 4 token tiles per core
QCH = 256          # query chunk
NQC = T // QCH     # 8 query chunks per batch
NKT = T // 128     # 16 key tiles per batch
EPS = 1e-5


def build(flags):
    (use_bq, use_bk, use_bv, use_bproj, use_bfc, use_bmlp,
     use_ln1wb, use_ln2wb, debug) = flags

    nc = bacc.Bacc("TRN2", target_bir_lowering=False, debug=False,
                   num_devices=CORES)

    # ---------------- DRAM parameters ----------------
    xt = nc.dram_tensor("xt", [C, S], BF16, kind="ExternalInput")
    w_attn = nc.dram_tensor("w_attn", [C, 3 * C], BF16, kind="ExternalInput")
    w_proj = nc.dram_tensor("w_proj", [C, C], BF16, kind="ExternalInput")
    w_fc = nc.dram_tensor("w_fc", [C, FF], BF16, kind="ExternalInput")
    w_mlp = nc.dram_tensor("w_mlp", [FF, C], BF16, kind="ExternalInput")
    b_q = nc.dram_tensor("b_q", [128, NCT], F32, kind="ExternalInput")
    b_k = nc.dram_tensor("b_k", [128, NCT], F32, kind="ExternalInput")
    b_v = nc.dram_tensor("b_v", [1, C], BF16, kind="ExternalInput")
    b_proj = nc.dram_tensor("b_proj", [128, NCT], F32, kind="ExternalInput")
    b_fc = nc.dram_tensor("b_fc", [128, NFT], F32, kind="ExternalInput")
    b_mlp = nc.dram_tensor("b_mlp", [128, NCT], F32, kind="ExternalInput")
    ln1w_d = nc.dram_tensor("ln1w", [128, NCT], F32, kind="ExternalInput")
    ln1b_d = nc.dram_tensor("ln1b", [128, NCT], F32, kind="ExternalInput")
    ln2w_d = nc.dram_tensor("ln2w", [128, NCT], F32, kind="ExternalInput")
    ln2b_d = nc.dram_tensor("ln2b", [128, NCT], F32, kind="ExternalInput")
    mask0_d = nc.dram_tensor("mask0", [128, 4 * QCH], BF16, kind="ExternalInput")
    mask1_d = nc.dram_tensor("mask1", [128, 4 * QCH], BF16, kind="ExternalInput")
    mask0x_d = nc.dram_tensor("mask0x", [128, 2 * QCH], BF16, kind="ExternalInput")
    mask1x_d = nc.dram_tensor("mask1x", [128, 2 * QCH], BF16, kind="ExternalInput")
    eye_d = nc.dram_tensor("eye", [128, 128], BF16, kind="ExternalInput")
    out_d = nc.dram_tensor("out", [C, S], BF16, kind="ExternalOutput")
    dbg = {}
    if debug:
        for nm, shp, dt in [("d_ln1", [C, S], BF16), ("d_qt", [C, S], BF16),
                            ("d_kt", [C, S], BF16), ("d_v", [S, C], BF16),
                            ("d_yt", [C, S], BF16), ("d_x2", [C, S], BF16),
                            ("d_ln2", [C, S], BF16)]:
            dbg[nm] = nc.dram_tensor(nm, shp, dt, kind="ExternalOutput")

    with tile.TileContext(nc) as tc:
        _build_body(nc, tc, locals(), flags)
    nc.compile()
    return nc


def _build_body(nc, tc, t_, flags):
    (use_bq, use_bk, use_bv, use_bproj, use_bfc, use_bmlp,
     use_ln1wb, use_ln2wb, debug) = flags
    xt = t_["xt"]
    w_attn, w_proj, w_fc, w_mlp = t_["w_attn"], t_["w_proj"], t_["w_fc"], t_["w_mlp"]
    b_q, b_k, b_v, b_proj, b_fc, b_mlp = (t_["b_q"], t_["b_k"], t_["b_v"],
                                          t_["b_proj"], t_["b_fc"], t_["b_mlp"])
    ln1w_d, ln1b_d, ln2w_d, ln2b_d = (t_["ln1w_d"], t_["ln1b_d"],
                                      t_["ln2w_d"], t_["ln2b_d"])
    mask0_d, mask1_d, out_d, dbg = t_["mask0_d"], t_["mask1_d"], t_["out_d"], t_["dbg"]
    mask0x_d, mask1x_d = t_["mask0x_d"], t_["mask1x_d"]
    eye_d = t_["eye_d"]

    from contextlib import ExitStack
    from concourse.tile import add_dep_helper

    def _delay_after(frm, to):
        # `to` (the DMA) waits on `frm` (the gate): first arg is the waiter
        f = frm.ins if hasattr(frm, "ins") else frm
        t = to.ins if hasattr(to, "ins") else to
        add_dep_helper(t, f, sync=True, reason="delay heavy DMA")
    es = ExitStack()

    consts = es.enter_context(tc.tile_pool(name="consts", bufs=1))
    dram = es.enter_context(tc.tile_pool(name="dram", bufs=1, space="DRAM"))

    # persistent pools
    xt_cm = tc.tile_pool(name="xt_p", bufs=1, side="right")
    xt_pool = xt_cm.__enter__()           # closed after proj phase
    x2t_p = es.enter_context(tc.tile_pool(name="x2t_p", bufs=1))
    fw_p = es.enter_context(tc.tile_pool(name="fw_p", bufs=1))
    # attention assembly tiles: allocated up-front so the v-pad memsets run
    # on the idle GpSimd at program start; closed after attention
    vtq_cm = tc.tile_pool(name="vtq_p", bufs=1)
    vtq = vtq_cm.__enter__()

    # ---- tiny consts (no DMA deps) ----
    ones_col = consts.tile([128, 1], BF16, name="ones_col")
    nc.vector.memset(ones_col, 1.0)
    ones_row = consts.tile([1, 128], BF16, name="ones_row")
    nc.vector.memset(ones_row, 1.0)
    eps_t = consts.tile([1, 1], F32, name="eps_t")
    nc.vector.memset(eps_t, EPS)
    warm_sb = consts.tile([1, 128], BF16, name="warm_sb")
    nc.vector.memset(warm_sb, 1.0)

    # warmup collective: absorbs the cc-stream cold-start cost right after
    # the cross-core rendezvous barrier, so the first real AllToAll runs at
    # full bandwidth
    cc0_in = dram.tile([1, 128], BF16, name="cc0_in")
    cc0_out = dram.tile([CORES, 128], BF16, name="cc0_out")
    nc.scalar.dma_start(out=cc0_in, in_=warm_sb)
    nc.gpsimd.collective_compute(
        "AllGather", ALU.bypass,
        replica_groups=[list(range(CORES))],
        ins=[cc0_in[:, :].opt()],
        outs=[cc0_out[:, :].opt()])

    qtb, ktb, vtb = [], [], {}
    for b in range(B):
        qtb.append(vtq.tile([128, T], FP8, name=f"qtb_{b}"))
        ktb.append(vtq.tile([128, T], FP8, name=f"ktb_{b}"))
        for r4 in range(4):
            # per (head, q4): col 0 = ones (denominator), 64:128 = V
            v3 = vtq.tile([128, 2, NTT, 128], BF16, name=f"vt_{b}_{r4}")
            nc.gpsimd.memset(v3[:, :, :, 0:1], 1.0)
            nc.gpsimd.memset(v3[:, :, :, 1:64], 0.0)
            vtb[(b, r4)] = v3

    # ---- collective DRAM tiles ----
    ccqk_in = dram.tile([CORES, 256, S], FP8, name="ccqk_in")
    ccqk_out = dram.tile([CORES, 256, S], FP8, name="ccqk_out")
    cc2_in = dram.tile([CORES, 2, S, 64], BF16, name="cc2_in")
    cc2_out = dram.tile([CORES, 2, S, 64], BF16, name="cc2_out")
    cc3_in = dram.tile([CORES, 128, S], BF16, name="cc3_in")
    cc3_out = dram.tile([CORES, 128, S], BF16, name="cc3_out")

    # =========================================================
    # bulk input streaming, priority order, on the SP DGE queue
    # =========================================================
    xt_sb = []
    for c in range(NCT):
        tl = xt_pool.tile([128, S], BF16, name=f"xt_{c}")
        nc.sync.dma_start(out=tl, in_=xt[c * 128:(c + 1) * 128, :])
        xt_sb.append(tl)

    # =========================================================
    # layernorm helpers (feature-major layout)
    # =========================================================
    def ln_stats(tag, pool, pspool, src_bf, c):
        """accumulate sum and sum-of-squares for tile c into psum row tiles"""
        if c == 0:
            s_ps = pspool.tile([1, S], F32, name=f"{tag}_sps", tag="st", bufs=2)
            q_ps = pspool.tile([1, S], F32, name=f"{tag}_qps", tag="st", bufs=2)
            ln_stats.st[tag] = (s_ps, q_ps)
        s_ps, q_ps = ln_stats.st[tag]
        sq = pool.tile([128, S], BF16, name=f"{tag}_sq_{c}",
                       tag=f"{tag}_sq", bufs=2)
        nc.vector.tensor_mul(sq, src_bf[c], src_bf[c])
        nc.tensor.matmul(s_ps[:, :], ones_col[:, :], src_bf[c][:, :],
                         start=(c == 0), stop=(c == NCT - 1))
        nc.tensor.matmul(q_ps[:, :], ones_col[:, :], sq[:, :],
                         start=(c == 0), stop=(c == NCT - 1))
        return (s_ps, q_ps)
    ln_stats.st = {}

    def bcast(pspool, tag, src_bf, n):
        """[1, n] bf16 row -> [128, n] f32 PSUM via rank-1 matmul."""
        ps = pspool.tile([128, 512], F32, name=f"{tag}_bc", tag="ps")
        nc.tensor.matmul(ps[:, :n], ones_row[:, :], src_bf[:, :n],
                         start=True, stop=True)
        return ps

    def ln_finish(tag, pool, pspool, src_bf, st, w_sb, b_sb, use_wb):
        s_ps, q_ps = st
        mu = pool.tile([1, S], F32, name=f"{tag}_mu")
        nc.scalar.mul(mu, s_ps[:, :], 1.0 / C)
        msq = pool.tile([1, S], F32, name=f"{tag}_msq")
        nc.scalar.mul(msq, q_ps[:, :], 1.0 / C)
        mu2 = pool.tile([1, S], F32, name=f"{tag}_mu2")
        nc.vector.tensor_mul(mu2, mu, mu)
        var = pool.tile([1, S], F32, name=f"{tag}_var")
        nc.vector.tensor_sub(var, msq, mu2)
        lnv = pool.tile([1, S], F32, name=f"{tag}_lnv")
        nc.scalar.activation(lnv, var, AF.Ln, bias=eps_t, scale=1.0)
        rstd = pool.tile([1, S], F32, name=f"{tag}_rstd")
        nc.scalar.activation(rstd, lnv, AF.Exp, scale=-0.5)
        rstd_bf = pool.tile([1, S], BF16, name=f"{tag}_rstd_bf")
        nc.vector.tensor_copy(rstd_bf, rstd)
        nmurs = pool.tile([1, S], F32, name=f"{tag}_nmurs")
        nc.vector.tensor_mul(nmurs, mu, rstd)
        nmurs_bf = pool.tile([1, S], BF16, name=f"{tag}_nmurs_bf")
        nc.scalar.mul(nmurs_bf, nmurs, -1.0)
        r_ps = bcast(pspool, f"{tag}_r", rstd_bf, S)
        sh_ps = bcast(pspool, f"{tag}_sh", nmurs_bf, S)
        r_b = pool.tile([128, S], BF16, name=f"{tag}_r_b")
        nc.scalar.copy(r_b, r_ps[:, :S])
        sh_b = pool.tile([128, S], BF16, name=f"{tag}_sh_b")
        nc.scalar.copy(sh_b, sh_ps[:, :S])
        outs = []
        for c in range(NCT):
            tmp = pool.tile([128, S], BF16, name=f"{tag}_tmp_{c}",
                            tag=f"{tag}_tmp", bufs=3)
            nc.vector.tensor_mul(tmp, src_bf[c], r_b)
            o = pool.tile([128, S], BF16, name=f"{tag}_o_{c}")
            if use_wb:
                nc.vector.tensor_add(tmp, tmp, sh_b)
                nc.vector.tensor_scalar(
                    out=o, in0=tmp,
                    scalar1=w_sb[:, c:c + 1], scalar2=b_sb[:, c:c + 1],
                    op0=ALU.mult, op1=ALU.add)
            else:
                nc.vector.tensor_add(o, tmp, sh_b)
            outs.append(o)
        return outs

    # =========================================================
    # Phase 1+2: LN1 and QKV projections (q, k, v weight chunks)
    # =========================================================
    ln1_pool = tc.tile_pool(name="ln1_pool", bufs=1)
    qkv_pool = tc.tile_pool(name="qkv_pool", bufs=1)
    psA_pool = tc.tile_pool(name="psA", bufs=6, space="PSUM")
    a2a_insts = {}
    with ln1_pool as lp, qkv_pool as qp, psA_pool as psA:
        # weight chunks, issued in consumption order on the SP queue
        aw_sb = {}
        for which, base in (("v", 2 * C), ("q", 0), ("k", C)):
            # v weights stream on the SP queue (with xt); q/k stream in
            # parallel on the Activation DGE queue so the PE isn't starved
            # waiting on a single ~150GB/s queue
            eng = nc.sync if which == "v" else nc.scalar
            for c in range(NCT):
                tl = lp.tile([128, C], BF16, name=f"aw_{which}_{c}",
                             tag="aw", bufs=16)
                eng.dma_start(out=tl,
                              in_=w_attn[c * 128:(c + 1) * 128,
                                         base:base + C])
                aw_sb[(which, c)] = tl
        # masks stream after the attention weights (needed only when the
        # attention loop starts)
        mask0 = consts.tile([128, 4 * QCH], BF16, name="mask0")
        nc.sync.dma_start(out=mask0, in_=mask0_d[:, :])
        mask1 = consts.tile([128, 4 * QCH], BF16, name="mask1")
        nc.sync.dma_start(out=mask1, in_=mask1_d[:, :])
        mask0x = consts.tile([128, 2 * QCH], BF16, name="mask0x")
        nc.sync.dma_start(out=mask0x, in_=mask0x_d[:, :])
        mask1x = consts.tile([128, 2 * QCH], BF16, name="mask1x")
        nc.sync.dma_start(out=mask1x, in_=mask1x_d[:, :])
        eye_sb = consts.tile([128, 128], BF16, name="eye_sb")
        nc.sync.dma_start(out=eye_sb, in_=eye_d[:, :])

        def load_const(name, dram_t, shape, dtype=F32):
            t = consts.tile(shape, dtype, name=name)
            nc.sync.dma_start(out=t, in_=dram_t[:, :])
            return t

        bq_sb = load_const("bq_sb", b_q, [128, NCT]) if use_bq else None
        bk_sb = load_const("bk_sb", b_k, [128, NCT]) if use_bk else None
        bv_sb = load_const("bv_sb", b_v, [1, C], BF16) if use_bv else None
        bproj_sb = load_const("bproj_sb", b_proj, [128, NCT]) if use_bproj else None
        bfc_sb = load_const("bfc_sb", b_fc, [128, NFT]) if use_bfc else None
        bmlp_sb = load_const("bmlp_sb", b_mlp, [128, NCT]) if use_bmlp else None
        ln1w_sb = load_const("ln1w_sb", ln1w_d, [128, NCT]) if use_ln1wb else None
        ln1b_sb = load_const("ln1b_sb", ln1b_d, [128, NCT]) if use_ln1wb else None
        ln2w_sb = load_const("ln2w_sb", ln2w_d, [128, NCT]) if use_ln2wb else None
        ln2b_sb = load_const("ln2b_sb", ln2b_d, [128, NCT]) if use_ln2wb else None

        # LN1
        for c in range(NCT):
            st1 = ln_stats("ln1", lp, psA, xt_sb, c)
        ln1t = ln_finish("ln1", lp, psA, xt_sb, st1, ln1w_sb, ln1b_sb,
                         use_ln1wb)
        if debug:
            for c in range(NCT):
                nc.sync.dma_start(out=dbg["d_ln1"][c * 128:(c + 1) * 128, :],
                                  in_=ln1t[c])

        # V, token-major, assembled in one SBUF tile then scattered
        # with one DMA per (destination block, head); V rides the FIRST
        # AllToAll because the attention loop needs it only a few kt steps
        # after the first scores, while q/k are needed immediately after --
        # so v transfers while the qk AllToAll still runs
        v_all = qp.tile([128, NTT, 2 * 512], BF16, name="v_all")
        for tt in range(NTT):
            for half in range(2):
                ps = psA.tile([128, 512], F32, name=f"vps_{tt}_{half}", tag="ps")
                for c in range(NCT):
                    nc.tensor.matmul(
                        ps[:, :],
                        ln1t[c][:, tt * 128:(tt + 1) * 128],
                        aw_sb[("v", c)][:, half * 512:(half + 1) * 512],
                        start=(c == 0), stop=(c == NCT - 1 and not use_bv))
                if use_bv:
                    nc.tensor.matmul(
                        ps[:, :], ones_row[:, :],
                        bv_sb[:, half * 512:(half + 1) * 512],
                        start=False, stop=True)
                nc.vector.tensor_copy(
                    v_all[:, tt, half * 512:(half + 1) * 512], ps[:, :])
                if debug:
                    o = qp.tile([128, 512], BF16, name=f"vdbg_{tt}_{half}",
                                tag="vdbg", bufs=2)
                    nc.vector.tensor_copy(o, ps[:, :])
                    nc.sync.dma_start(
                        out=dbg["d_v"][tt * 128:(tt + 1) * 128,
                                       half * 512:(half + 1) * 512],
                        in_=o)
        for j in range(CORES):
            for h in range(2):
                nc.scalar.dma_start(
                    out=cc2_in[j, h].rearrange("(a p) f -> p a f", p=128),
                    in_=v_all[:, :, j * 128 + h * 64:j * 128 + (h + 1) * 64])

        a2a_insts["v"] = nc.gpsimd.collective_compute(
            "AllToAll", ALU.bypass,
            replica_groups=[list(range(CORES))],
            ins=[cc2_in[:, :, :].opt()],
            outs=[cc2_out[:, :, :].opt()])

        # Q^T and K^T, feature-major [C, S], into ONE merged AllToAll
        for which, bias_sb, useb, row0 in (
                ("q", bq_sb, use_bq, 0), ("k", bk_sb, use_bk, 128)):
            for hp in range(NCT):
                ps = psA.tile([128, 512], F32, name=f"{which}ps_{hp}", tag="ps")
                for c in range(NCT):
                    nc.tensor.matmul(
                        ps[:, :],
                        aw_sb[(which, c)][:, hp * 128:(hp + 1) * 128],
                        ln1t[c][:, :],
                        start=(c == 0), stop=(c == NCT - 1))
                o = qp.tile([128, S], FP8, name=f"{which}t_{hp}",
                            tag=f"{which}t", bufs=2)
                if useb:
                    nc.vector.tensor_scalar_add(o, ps[:, :],
                                                bias_sb[:, hp:hp + 1])
                else:
                    nc.vector.tensor_copy(o, ps[:, :])
                nc.scalar.dma_start(out=ccqk_in[hp, row0:row0 + 128, :],
                                    in_=o)
                if debug:
                    nm = "d_qt" if which == "q" else "d_kt"
                    nc.sync.dma_start(out=dbg[nm][hp * 128:(hp + 1) * 128, :],
                                      in_=o)
        a2a_insts["qk"] = nc.gpsimd.collective_compute(
            "AllToAll", ALU.bypass,
            replica_groups=[list(range(CORES))],
            ins=[ccqk_in[:, :, :].opt()],
            outs=[ccqk_out[:, :, :].opt()])

    # proj weights: prefetch during attention (SP queue, after masks)
    fw_dmas = {}
    pw_sb = []
    for c in range(NCT):
        tl = fw_p.tile([128, C], BF16, name=f"pw_{c}")
        d = nc.sync.dma_start(out=tl, in_=w_proj[c * 128:(c + 1) * 128, :])
        fw_dmas[("pw", c)] = d
        pw_sb.append(tl)

    # fc weights: quarter-slabs [128, 1024] with a 24-slot rotation (three
    # quarters resident). Allocated (and DMAs issued) before the attention
    # pool opens so the SBUF zone is fresh (no WAR deps on attention tiles);
    # transfers are gated onto attention-phase anchors below, streaming
    # through the attention window when DMA is otherwise idle. Quarter 3's
    # slots free as quarter 0 is consumed in the fc loop.
    fw_sb = {}
    for quarter in range(4):
        for c in range(NCT):
            tl = fw_p.tile([128, FF // 4], BF16, name=f"fw_{quarter}_{c}",
                           tag="fw", bufs=24)
            fw_dmas[(quarter, c)] = nc.sync.dma_start(
                out=tl,
                in_=w_fc[c * 128:(c + 1) * 128,
                         quarter * (FF // 4):(quarter + 1) * (FF // 4)])
            fw_sb[(quarter, c)] = tl

    # =========================================================
    # Phase 3: attention (my 2 heads, all tokens)
    # =========================================================
    att_pool = tc.tile_pool(name="att_pool", bufs=1)
    yta_pool = tc.tile_pool(name="yta_pool", bufs=1)
    psB_pool = tc.tile_pool(name="psB", bufs=2, space="PSUM")
    with att_pool as ap, psB_pool as psB:
        for b in range(B):
            nc.scalar.dma_start(
                out=qtb[b][:, :].rearrange("p (r s) -> p r s", r=4),
                in_=ccqk_out[4 * b:4 * b + 4, 0:128, :].rearrange(
                    "r p s -> p r s"))
            nc.scalar.dma_start(
                out=ktb[b][:, :].rearrange("p (r s) -> p r s", r=4),
                in_=ccqk_out[4 * b:4 * b + 4, 128:256, :].rearrange(
                    "r p s -> p r s"))
            for r4 in range(4):
                nc.gpsimd.dma_start(
                    out=vtb[(b, r4)][:, :, :, 64:128],
                    in_=cc2_out[4 * b + r4].rearrange(
                        "h (a p) f -> p h a f", p=128))

        anchors = {}
        pending = []

        def flush_normalize(item):
            fb, fp, y_A, y_B, ytAB = item
            for hh, y_ps in enumerate((y_A, y_B)):
                rec = ap.tile([1, W2], F32, name=f"rec_{fb}_{fp}_{hh}",
                              tag="rec", bufs=4)
                nc.vector.reciprocal_approx_fast(rec, y_ps[0:1, :])
                den = ap.tile([64, W2], F32, name=f"den_{fb}_{fp}_{hh}",
                              tag="den", bufs=4)
                nc.gpsimd.partition_broadcast(den, rec)
                nc.vector.tensor_mul(ytAB[hh * 64:(hh + 1) * 64, :],
                                     y_ps[64:128, :], den)
            nc.gpsimd.dma_start(out=cc3_in[4 * fb + fp], in_=ytAB)
        # process query chunks in PAIRS (qc, qc+1): shared key tiles get one
        # N=512 matmul covering both chunks' queries; the pair's last two key
        # tiles (diagonal of chunk qc+1) run N=256 on chunk qc+1 only.
        W2 = 2 * QCH
        npair = 0
        for b in range(B):
            for p in reversed(range(NQC // 2)):
                qc = 2 * p
                qs = qc * QCH
                nsh = 2 * (qc + 1)          # shared key tiles
                # y accumulators: rows 0=den, 64:128=y; cols = 2 chunks
                y_A = psB.tile([128, W2], F32, name=f"yA_{b}_{p}", tag="ya",
                               bufs=4)
                y_B = psB.tile([128, W2], F32, name=f"yB_{b}_{p}", tag="ya",
                               bufs=4)
                # normalized y (head A rows 0:64, head B rows 64:128)
                ytAB = ap.tile([128, W2], BF16, name=f"ytab_{b}_{p}",
                               tag="ytAB", bufs=4)
                flush_due = pending.pop(0) if pending else None
                for kt in range(nsh + 2):
                    if kt == 1 and flush_due is not None:
                        flush_normalize(flush_due)
                        flush_due = None
                    shared = kt < nsh
                    cols = slice(0, W2) if shared else slice(QCH, W2)
                    ncols = W2 if shared else QCH
                    # scores for both heads into one 2-bank PSUM tile
                    s_AB = psB.tile([128, 2 * W2], F32, name=f"s_{b}_{p}_{kt}",
                                    tag="ps2", bufs=2)
                    nc.tensor.matmul(s_AB[:, 0:ncols],
                                     ktb[b][0:64, kt * 128:(kt + 1) * 128],
                                     qtb[b][0:64, qs + cols.start:qs + W2],
                                     start=True, stop=True)
                    nc.tensor.matmul(s_AB[:, W2:W2 + ncols],
                                     ktb[b][64:128, kt * 128:(kt + 1) * 128],
                                     qtb[b][64:128, qs + cols.start:qs + W2],
                                     start=True, stop=True)
                    e_AB = ap.tile([128, 2 * W2], BF16, name=f"e_{b}_{p}_{kt}",
                                   tag="eAB", bufs=8)
                    if shared:
                        nc.scalar.activation(e_AB, s_AB[:, :], AF.Exp,
                                             scale=1.0 / np.sqrt(HS))
                        if kt == qc * 2:
                            nc.vector.tensor_mul(e_AB, e_AB, mask0)
                        elif kt == qc * 2 + 1:
                            nc.vector.tensor_mul(e_AB, e_AB, mask1)
                    else:
                        e3 = e_AB.rearrange("p (h q) -> p h q", h=2)
                        s3 = s_AB.rearrange("p (h q) -> p h q", h=2)
                        nc.scalar.activation(e3[:, :, 0:QCH], s3[:, :, 0:QCH],
                                             AF.Exp, scale=1.0 / np.sqrt(HS))
                        mx = mask0x if kt == nsh else mask1x
                        nc.vector.tensor_mul(
                            e_AB.rearrange("p (h q) -> p h q", h=2)[:, :, 0:QCH],
                            e_AB.rearrange("p (h q) -> p h q", h=2)[:, :, 0:QCH],
                            mx.rearrange("p (h q) -> p h q", h=2))
                    v3 = vtb[(b, kt // 4)]
                    q4 = kt % 4
                    nc.tensor.matmul(y_A[:, cols], v3[:, 0, q4, :],
                                     e_AB[:, 0:ncols],
                                     start=(kt == 0), stop=(kt == nsh + 1),
                                     skip_group_check=True)
                    mmB = nc.tensor.matmul(y_B[:, cols], v3[:, 1, q4, :],
                                           e_AB[:, W2:W2 + ncols],
                                           start=(kt == 0),
                                           stop=(kt == nsh + 1),
                                           skip_group_check=True)
                    if kt == nsh + 1:
                        anchors[npair] = mmB
                # normalize is deferred one pair (flushed inside the NEXT
                # pair's kt loop) so its vector ops interleave mid-stream
                pending.append((b, p, y_A, y_B, ytAB))
                npair += 1
        while pending:
            flush_normalize(pending.pop(0))
        a2a_y = nc.gpsimd.collective_compute(
            "AllToAll", ALU.bypass,
            replica_groups=[list(range(CORES))],
            ins=[cc3_in[:, :, :].opt()],
            outs=[cc3_out[:, :, :].opt()])

    vtq_cm.__exit__(None, None, None)  # free qtb/ktb/vt SBUF

    # now that anchors exist, gate the fc weight stream onto them
    # (quarter 3 is additionally slot-gated on quarter 0's consumption)
    for c in range(NCT):
        _delay_after(anchors[0], fw_dmas[("pw", c)])
    for quarter in range(4):
        gate = anchors[[0, 1, 2, 4][quarter]]
        for c in range(NCT):
            _delay_after(gate, fw_dmas[(quarter, c)])

    # =========================================================
    # Phase 4: proj + residual (+ interleaved LN2 stats)
    # =========================================================
    mlp_pool = tc.tile_pool(name="mlp_pool", bufs=1)
    psC_cm = tc.tile_pool(name="psC", bufs=6, space="PSUM")
    psC = psC_cm.__enter__()
    with yta_pool as yp, mlp_pool as mp:
        yta_big = yp.tile([128, NCT, S], BF16, name="yta_big")
        nc.gpsimd.dma_start(out=yta_big,
                            in_=cc3_out[:, :, :].rearrange("h p s -> p h s"))
        yta = [yta_big[:, hp, :] for hp in range(NCT)]
        if debug:
            for hp in range(NCT):
                nc.sync.dma_start(out=dbg["d_yt"][hp * 128:(hp + 1) * 128, :],
                                  in_=yta[hp])
        # keep the PE clock ramped through the y-AllToAll wait: a chain of
        # dependency-free rank-1 matmuls runs back-to-back in the gap (the
        # following proj matmuls are data-gated on yta)
        warm_ps = psC.tile([128, 512], F32, name="warm_ps", tag="ps")
        for w in range(115):
            nc.tensor.matmul(warm_ps[0:1, :], ones_col[:, :],
                             pw_sb[0][:, 0:512], start=True, stop=True,
                             skip_group_check=True)
        x2bf_sb = []
        for co in range(NCT):
            ps = psC.tile([128, 512], F32, name=f"prps_{co}", tag="ps")
            for ci in range(NCT):
                nc.tensor.matmul(ps[:, :],
                                 pw_sb[ci][:, co * 128:(co + 1) * 128],
                                 yta[ci],
                                 start=(ci == 0), stop=False)
            # residual add on the PE: accumulate eye @ x (exact in bf16)
            nc.tensor.matmul(ps[:, :], eye_sb[:, :], xt_sb[co][:, :],
                             start=False, stop=True)
            x2b = x2t_p.tile([128, S], BF16, name=f"x2bf_{co}")
            if use_bproj:
                nc.vector.tensor_scalar_add(x2b, ps[:, :],
                                            bproj_sb[:, co:co + 1])
            else:
                nc.scalar.copy(x2b, ps[:, :])
            x2bf_sb.append(x2b)
            # LN2 stats ride along so the finish chain starts immediately
            st2 = ln_stats("ln2", mp, psC, x2bf_sb, co)
            if debug:
                nc.sync.dma_start(out=dbg["d_x2"][co * 128:(co + 1) * 128, :],
                                  in_=x2b)
        xt_cm.__exit__(None, None, None)  # free xt SBUF

        # Phase 5: LN2 finish
        ln2t = ln_finish("ln2", mp, psC, x2bf_sb, st2, ln2w_sb, ln2b_sb,
                         use_ln2wb)
        if debug:
            for c in range(NCT):
                nc.sync.dma_start(out=dbg["d_ln2"][c * 128:(c + 1) * 128, :],
                                  in_=ln2t[c])

        # Phase 6: fc + GELU
        ht = []
        for f in range(NFT):
            quarter, fo = f // (NFT // 4), f % (NFT // 4)
            ps = psC.tile([128, 512], F32, name=f"fcps_{f}", tag="ps")
            for c in range(NCT):
                nc.tensor.matmul(ps[:, :],
                                 fw_sb[(quarter, c)][:, fo * 128:(fo + 1) * 128],
                                 ln2t[c][:, :],
                                 start=(c == 0), stop=(c == NCT - 1))
            h = mp.tile([128, S], BF16, name=f"ht_{f}")
            if use_bfc:
                nc.scalar.activation(h, ps[:, :], AF.Gelu,
                                     bias=bfc_sb[:, f:f + 1], scale=1.0)
            else:
                nc.scalar.activation(h, ps[:, :], AF.Gelu, scale=1.0)
            ht.append(h)

        # Phase 7: mlp proj + residual -> out (f-outer, 8 PSUM accumulators)
        psC_cm.__exit__(None, None, None)
        psM_cm = tc.tile_pool(name="psM", bufs=8, space="PSUM")
        psM = psM_cm.__enter__()
        accs = [psM.tile([128, 512], F32, name=f"mlps_{co}", tag="psm",
                         bufs=8) for co in range(NCT)]
        # f-outer for the first 24 hidden tiles (weights stream through a
        # rotating pool); the last 8 run co-outer so the output slabs finish
        # STAGGERED and their PSUM copy + store overlap the remaining matmuls
        NTAIL = 8
        mw_tail = []
        for f in range(NFT):
            tl = mp.tile([128, C], BF16, name=f"mw_{f}", tag="mw", bufs=12)
            d = nc.sync.dma_start(out=tl, in_=w_mlp[f * 128:(f + 1) * 128, :])
            _delay_after(anchors[6], d)
            if f < NFT - NTAIL:
                for co in range(NCT):
                    nc.tensor.matmul(accs[co][:, :],
                                     tl[:, co * 128:(co + 1) * 128],
                                     ht[f][:, :],
                                     start=(f == 0), stop=False)
            else:
                mw_tail.append((f, tl))
        for co in range(NCT):
            for f, tl in mw_tail:
                nc.tensor.matmul(accs[co][:, :],
                                 tl[:, co * 128:(co + 1) * 128],
                                 ht[f][:, :], start=False, stop=False)
            # residual add on the PE, then a scalar-engine copy out of PSUM
            nc.tensor.matmul(accs[co][:, :], eye_sb[:, :],
                             x2bf_sb[co][:, :], start=False, stop=True)
            o = mp.tile([128, S], BF16, name=f"out_{co}", tag="outt", bufs=3)
            if use_bmlp:
                nc.vector.tensor_scalar_add(o, accs[co][:, :],
                                            bmlp_sb[:, co:co + 1])
            else:
                nc.scalar.copy(o, accs[co][:, :])
            nc.scalar.dma_start(out=out_d[co * 128:(co + 1) * 128, :], in_=o)
        psM_cm.__exit__(None, None, None)

    es.close()


# =============================================================
# Host side
# =============================================================
_CACHE = {}


def _get_nc(flags):
    if flags not in _CACHE:
        _CACHE[flags] = build(flags)
    return _CACHE[flags]


def _prep(inputs, debug=False):
    f32 = np.float32
    x = np.asarray(inputs["x"], f32)
    attn_w = np.asarray(inputs["attn_w"], f32)
    attn_b = np.asarray(inputs["attn_b"], f32)
    proj_w = np.asarray(inputs["proj_w"], f32)
    proj_b = np.asarray(inputs["proj_b"], f32)
    fc_w = np.asarray(inputs["fc_w"], f32)
    fc_b = np.asarray(inputs["fc_b"], f32)
    mlp_w = np.asarray(inputs["mlp_proj_w"], f32)
    mlp_b = np.asarray(inputs["mlp_proj_b"], f32)
    ln1w = np.asarray(inputs["ln1_w"], f32)
    ln1b = np.asarray(inputs["ln1_b"], f32)
    ln2w = np.asarray(inputs["ln2_w"], f32)
    ln2b = np.asarray(inputs["ln2_b"], f32)

    def nz(a):
        return bool(np.any(a != 0.0))

    flags = (nz(attn_b[0:C]), nz(attn_b[C:2 * C]), nz(attn_b[2 * C:3 * C]),
             nz(proj_b), nz(fc_b), nz(mlp_b),
             nz(ln1w - 1.0) or nz(ln1b), nz(ln2w - 1.0) or nz(ln2b), debug)

    def colsplit(v):
        # [n*128] -> [128, n] with col j = v[j*128:(j+1)*128]
        return np.ascontiguousarray(v.reshape(-1, 128).T)

    bf = lambda a: np.ascontiguousarray(a).astype(NP_BF16)
    shared = {
        "w_attn": bf(attn_w), "w_proj": bf(proj_w), "w_fc": bf(fc_w),
        "w_mlp": bf(mlp_w),
        "b_q": colsplit(attn_b[0:C]), "b_k": colsplit(attn_b[C:2 * C]),
        "b_v": bf(attn_b[2 * C:3 * C].reshape(1, C)),
        "b_proj": colsplit(proj_b), "b_fc": colsplit(fc_b),
        "b_mlp": colsplit(mlp_b),
        "ln1w": colsplit(ln1w), "ln1b": colsplit(ln1b),
        "ln2w": colsplit(ln2w), "ln2b": colsplit(ln2b),
    }
    k_idx = np.arange(128)[:, None]
    q_idx = np.arange(QCH)[None, :]
    m0 = (q_idx >= k_idx).astype(NP_BF16)
    m1 = (q_idx >= k_idx + 128).astype(NP_BF16)
    one = np.ones((128, QCH), NP_BF16)
    shared["mask0"] = np.concatenate([m0, one, m0, one], axis=1)
    shared["mask1"] = np.concatenate([m1, one, m1, one], axis=1)
    shared["mask0x"] = np.concatenate([m0, m0], axis=1)
    shared["mask1x"] = np.concatenate([m1, m1], axis=1)
    shared["eye"] = np.eye(128, dtype=NP_BF16)

    in_maps = []
    for i in range(CORES):
        b, s = i // 4, i % 4
        xs = np.ascontiguousarray(x[b, s * S:(s + 1) * S, :].T)  # [C, S]
        m = dict(shared)
        m["xt"] = xs.astype(NP_BF16)
        in_maps.append(m)
    return flags, in_maps


def run_sharded(inputs, debug=False, trace=False, trace_kwargs=None):
    flags, in_maps = _prep(inputs, debug)
    nc = _get_nc(flags)
    res = bass_utils.run_bass_kernel_spmd(
        nc, in_maps, core_ids=list(range(CORES)), trace=trace,
        **(trace_kwargs or {}))
    out = np.empty((B, T, C), np.float32)
    for i in range(CORES):
        b, s = i // 4, i % 4
        out[b, s * S:(s + 1) * S, :] = np.asarray(
            res.results[i]["out"], np.float32).T
    return out, res


def kernel(**inputs):
    out, _ = run_sharded(inputs, debug=False, trace=False)
    return out


# revision 40
# speedup vs baseline: 1.0458x; 1.0458x over previous
"""Trainium2 8-core kernel for a GPT-style transformer block.

Strategy:
  - Token-parallel everywhere except attention: each core owns a contiguous
    512-token span (core i -> batch i//4, span i%4). LayerNorms, QKV, proj,
    MLP and residuals are purely local to the core's tokens.
  - Attention is head-parallel: one AllToAll redistributes Q^T/K^T together
    (feature-major) and one moves V (token-major) so core j holds head-pair
    j (heads 2j, 2j+1) for ALL 4096 tokens. Every core runs the identical
    causal-attention loop (SPMD-uniform). A third AllToAll brings y back
    token-sharded.
  - Activations are kept feature-major [C, tokens] so every matmul contraction
    sits on the partition axis; LN statistics are computed with ones-vector
    matmuls on the TensorEngine and broadcast back with rank-1 matmuls.
  - Compute dtype bf16 (fp32 PSUM accumulation); residual stream fp32 in SBUF
    sourced from the bf16 x input.
  - Softmax skips max-subtraction (scores are O(1) for this problem's scale)
    and gets denominators for free via a ones-column prepended to V; the
    per-query normalization uses the fast DVE reciprocal (no scalar-engine
    ln/exp -> no activation-table thrash against the attention exps).
  - Scheduling: bulk weight streaming rides the SP DGE queue in priority
    order (xt, attn-q/k/v chunks, masks, then gated proj/fc/mlp prefetch
    that streams through the attention window when DMA is otherwise idle,
    keeping the y AllToAll uncontended). cc-buffer writes and attention
    assembly loads ride the Activation DGE queue (batched, few
    instructions); flush/output DMAs ride the GpSimd SWDGE.
"""

import sys

sys.path.insert(0, "/opt/trn_rl_repo")

import numpy as np
import ml_dtypes

import concourse.bass as bass
import concourse.mybir as mybir
import concourse.tile as tile
from concourse import bacc, bass_utils

BF16 = mybir.dt.bfloat16
FP8 = mybir.dt.float8e4
F32 = mybir.dt.float32
AF = mybir.ActivationFunctionType
ALU = mybir.AluOpType
NP_BF16 = ml_dtypes.bfloat16

B, T, C, H, HS, FF = 2, 2048, 1024, 16, 64, 4096
CORES = 8
S = 512            # tokens per core
NCT = C // 128     # 8 feature tiles
NFT = FF // 128    # 32 mlp hidden tiles
NTT = S // 128     # 4 token tiles per core
QCH = 256          # query chunk
NQC = T // QCH     # 8 query chunks per batch
NKT = T // 128     # 16 key tiles per batch
EPS = 1e-5


def build(flags):
    (use_bq, use_bk, use_bv, use_bproj, use_bfc, use_bmlp,
     use_ln1wb, use_ln2wb, debug) = flags

    nc = bacc.Bacc("TRN2", target_bir_lowering=False, debug=False,
                   num_devices=CORES)

    # ---------------- DRAM parameters ----------------
    xt = nc.dram_tensor("xt", [C, S], BF16, kind="ExternalInput")
    w_attn = nc.dram_tensor("w_attn", [C, 3 * C], BF16, kind="ExternalInput")
    w_proj = nc.dram_tensor("w_proj", [C, C], BF16, kind="ExternalInput")
    w_fc = nc.dram_tensor("w_fc", [C, FF], BF16, kind="ExternalInput")
    w_mlp = nc.dram_tensor("w_mlp", [FF, C], BF16, kind="ExternalInput")
    b_q = nc.dram_tensor("b_q", [128, NCT], F32, kind="ExternalInput")
    b_k = nc.dram_tensor("b_k", [128, NCT], F32, kind="ExternalInput")
    b_v = nc.dram_tensor("b_v", [1, C], BF16, kind="ExternalInput")
    b_proj = nc.dram_tensor("b_proj", [128, NCT], F32, kind="ExternalInput")
    b_fc = nc.dram_tensor("b_fc", [128, NFT], F32, kind="ExternalInput")
    b_mlp = nc.dram_tensor("b_mlp", [128, NCT], F32, kind="ExternalInput")
    ln1w_d = nc.dram_tensor("ln1w", [128, NCT], F32, kind="ExternalInput")
    ln1b_d = nc.dram_tensor("ln1b", [128, NCT], F32, kind="ExternalInput")
    ln2w_d = nc.dram_tensor("ln2w", [128, NCT], F32, kind="ExternalInput")
    ln2b_d = nc.dram_tensor("ln2b", [128, NCT], F32, kind="ExternalInput")
    mask0_d = nc.dram_tensor("mask0", [128, 4 * QCH], BF16, kind="ExternalInput")
    mask1_d = nc.dram_tensor("mask1", [128, 4 * QCH], BF16, kind="ExternalInput")
    mask0x_d = nc.dram_tensor("mask0x", [128, 2 * QCH], BF16, kind="ExternalInput")
    mask1x_d = nc.dram_tensor("mask1x", [128, 2 * QCH], BF16, kind="ExternalInput")
    eye_d = nc.dram_tensor("eye", [128, 128], BF16, kind="ExternalInput")
    out_d = nc.dram_tensor("out", [C, S], BF16, kind="ExternalOutput")
    dbg = {}
    if debug:
        for nm, shp, dt in [("d_ln1", [C, S], BF16), ("d_qt", [C, S], BF16),
                            ("d_kt", [C, S], BF16), ("d_v", [S, C], BF16),
                            ("d_yt", [C, S], BF16), ("d_x2", [C, S], BF16),
                            ("d_ln2", [C, S], BF16)]:
            dbg[nm] = nc.dram_tensor(nm, shp, dt, kind="ExternalOutput")

    with tile.TileContext(nc) as tc:
        _build_body(nc, tc, locals(), flags)
    nc.compile()
    return nc


def _build_body(nc, tc, t_, flags):
    (use_bq, use_bk, use_bv, use_bproj, use_bfc, use_bmlp,
     use_ln1wb, use_ln2wb, debug) = flags
    xt = t_["xt"]
    w_attn, w_proj, w_fc, w_mlp = t_["w_attn"], t_["w_proj"], t_["w_fc"], t_["w_mlp"]
    b_q, b_k, b_v, b_proj, b_fc, b_mlp = (t_["b_q"], t_["b_k"], t_["b_v"],
                                          t_["b_proj"], t_["b_fc"], t_["b_mlp"])
    ln1w_d, ln1b_d, ln2w_d, ln2b_d = (t_["ln1w_d"], t_["ln1b_d"],
                                      t_["ln2w_d"], t_["ln2b_d"])
    mask0_d, mask1_d, out_d, dbg = t_["mask0_d"], t_["mask1_d"], t_["out_d"], t_["dbg"]
    mask0x_d, mask1x_d = t_["mask0x_d"], t_["mask1x_d"]
    eye_d = t_["eye_d"]

    from contextlib import ExitStack
    from concourse.tile import add_dep_helper

    def _delay_after(frm, to):
        # `to` (the DMA) waits on `frm` (the gate): first arg is the waiter
        f = frm.ins if hasattr(frm, "ins") else frm
        t = to.ins if hasattr(to, "ins") else to
        add_dep_helper(t, f, sync=True, reason="delay heavy DMA")
    es = ExitStack()

    consts = es.enter_context(tc.tile_pool(name="consts", bufs=1))
    dram = es.enter_context(tc.tile_pool(name="dram", bufs=1, space="DRAM"))

    # persistent pools
    xt_cm = tc.tile_pool(name="xt_p", bufs=1, side="right")
    xt_pool = xt_cm.__enter__()           # closed after proj phase
    x2t_p = es.enter_context(tc.tile_pool(name="x2t_p", bufs=1))
    fw_p = es.enter_context(tc.tile_pool(name="fw_p", bufs=1))
    # attention assembly tiles: allocated up-front so the v-pad memsets run
    # on the idle GpSimd at program start; closed after attention
    vtq_cm = tc.tile_pool(name="vtq_p", bufs=1)
    vtq = vtq_cm.__enter__()

    # ---- tiny consts (no DMA deps) ----
    ones_col = consts.tile([128, 1], BF16, name="ones_col")
    nc.vector.memset(ones_col, 1.0)
    ones_row = consts.tile([1, 128], BF16, name="ones_row")
    nc.vector.memset(ones_row, 1.0)
    eps_t = consts.tile([1, 1], F32, name="eps_t")
    nc.vector.memset(eps_t, EPS)
    warm_sb = consts.tile([1, 128], BF16, name="warm_sb")
    nc.vector.memset(warm_sb, 1.0)

    # warmup collective: absorbs the cc-stream cold-start cost right after
    # the cross-core rendezvous barrier, so the first real AllToAll runs at
    # full bandwidth
    cc0_in = dram.tile([1, 128], BF16, name="cc0_in")
    cc0_out = dram.tile([CORES, 128], BF16, name="cc0_out")
    nc.scalar.dma_start(out=cc0_in, in_=warm_sb)
    nc.gpsimd.collective_compute(
        "AllGather", ALU.bypass,
        replica_groups=[list(range(CORES))],
        ins=[cc0_in[:, :].opt()],
        outs=[cc0_out[:, :].opt()])

    qtb, ktb, vtb = [], [], {}
    for b in range(B):
        qtb.append(vtq.tile([128, T], FP8, name=f"qtb_{b}"))
        ktb.append(vtq.tile([128, T], FP8, name=f"ktb_{b}"))
        for r4 in range(4):
            # per (head, q4): col 0 = ones (denominator), 64:128 = V
            v3 = vtq.tile([128, 2, NTT, 128], BF16, name=f"vt_{b}_{r4}")
            nc.gpsimd.memset(v3[:, :, :, 0:1], 1.0)
            nc.gpsimd.memset(v3[:, :, :, 1:64], 0.0)
            vtb[(b, r4)] = v3

    # ---- collective DRAM tiles ----
    ccqk_in = dram.tile([CORES, 256, S], FP8, name="ccqk_in")
    ccqk_out = dram.tile([CORES, 256, S], FP8, name="ccqk_out")
    cc2_in = dram.tile([CORES, 2, S, 64], BF16, name="cc2_in")
    cc2_out = dram.tile([CORES, 2, S, 64], BF16, name="cc2_out")
    cc3_in = dram.tile([CORES, 128, S], BF16, name="cc3_in")
    cc3_out = dram.tile([CORES, 128, S], BF16, name="cc3_out")

    # =========================================================
    # bulk input streaming, priority order, on the SP DGE queue
    # =========================================================
    xt_sb = []
    for c in range(NCT):
        tl = xt_pool.tile([128, S], BF16, name=f"xt_{c}")
        nc.sync.dma_start(out=tl, in_=xt[c * 128:(c + 1) * 128, :])
        xt_sb.append(tl)

    # =========================================================
    # layernorm helpers (feature-major layout)
    # =========================================================
    def ln_stats(tag, pool, pspool, src_bf, c):
        """accumulate sum and sum-of-squares for tile c into psum row tiles"""
        if c == 0:
            s_ps = pspool.tile([1, S], F32, name=f"{tag}_sps", tag="st", bufs=2)
            q_ps = pspool.tile([1, S], F32, name=f"{tag}_qps", tag="st", bufs=2)
            ln_stats.st[tag] = (s_ps, q_ps)
        s_ps, q_ps = ln_stats.st[tag]
        sq = pool.tile([128, S], BF16, name=f"{tag}_sq_{c}",
                       tag=f"{tag}_sq", bufs=2)
        nc.vector.tensor_mul(sq, src_bf[c], src_bf[c])
        nc.tensor.matmul(s_ps[:, :], ones_col[:, :], src_bf[c][:, :],
                         start=(c == 0), stop=(c == NCT - 1))
        nc.tensor.matmul(q_ps[:, :], ones_col[:, :], sq[:, :],
                         start=(c == 0), stop=(c == NCT - 1))
        return (s_ps, q_ps)
    ln_stats.st = {}

    def bcast(pspool, tag, src_bf, n):
        """[1, n] bf16 row -> [128, n] f32 PSUM via rank-1 matmul."""
        ps = pspool.tile([128, 512], F32, name=f"{tag}_bc", tag="ps")
        nc.tensor.matmul(ps[:, :n], ones_row[:, :], src_bf[:, :n],
                         start=True, stop=True)
        return ps

    def ln_finish(tag, pool, pspool, src_bf, st, w_sb, b_sb, use_wb):
        s_ps, q_ps = st
        mu = pool.tile([1, S], F32, name=f"{tag}_mu")
        nc.scalar.mul(mu, s_ps[:, :], 1.0 / C)
        msq = pool.tile([1, S], F32, name=f"{tag}_msq")
        nc.scalar.mul(msq, q_ps[:, :], 1.0 / C)
        mu2 = pool.tile([1, S], F32, name=f"{tag}_mu2")
        nc.vector.tensor_mul(mu2, mu, mu)
        var = pool.tile([1, S], F32, name=f"{tag}_var")
        nc.vector.tensor_sub(var, msq, mu2)
        lnv = pool.tile([1, S], F32, name=f"{tag}_lnv")
        nc.scalar.activation(lnv, var, AF.Ln, bias=eps_t, scale=1.0)
        rstd = pool.tile([1, S], F32, name=f"{tag}_rstd")
        nc.scalar.activation(rstd, lnv, AF.Exp, scale=-0.5)
        rstd_bf = pool.tile([1, S], BF16, name=f"{tag}_rstd_bf")
        nc.vector.tensor_copy(rstd_bf, rstd)
        nmurs = pool.tile([1, S], F32, name=f"{tag}_nmurs")
        nc.vector.tensor_mul(nmurs, mu, rstd)
        nmurs_bf = pool.tile([1, S], BF16, name=f"{tag}_nmurs_bf")
        nc.scalar.mul(nmurs_bf, nmurs, -1.0)
        r_ps = bcast(pspool, f"{tag}_r", rstd_bf, S)
        sh_ps = bcast(pspool, f"{tag}_sh", nmurs_bf, S)
        r_b = pool.tile([128, S], BF16, name=f"{tag}_r_b")
        nc.scalar.copy(r_b, r_ps[:, :S])
        sh_b = pool.tile([128, S], BF16, name=f"{tag}_sh_b")
        nc.scalar.copy(sh_b, sh_ps[:, :S])
        outs = []
        for c in range(NCT):
            tmp = pool.tile([128, S], BF16, name=f"{tag}_tmp_{c}",
                            tag=f"{tag}_tmp", bufs=3)
            nc.vector.tensor_mul(tmp, src_bf[c], r_b)
            o = pool.tile([128, S], BF16, name=f"{tag}_o_{c}")
            if use_wb:
                nc.vector.tensor_add(tmp, tmp, sh_b)
                nc.vector.tensor_scalar(
                    out=o, in0=tmp,
                    scalar1=w_sb[:, c:c + 1], scalar2=b_sb[:, c:c + 1],
                    op0=ALU.mult, op1=ALU.add)
            else:
                nc.vector.tensor_add(o, tmp, sh_b)
            outs.append(o)
        return outs

    # =========================================================
    # Phase 1+2: LN1 and QKV projections (q, k, v weight chunks)
    # =========================================================
    ln1_pool = tc.tile_pool(name="ln1_pool", bufs=1)
    qkv_pool = tc.tile_pool(name="qkv_pool", bufs=1)
    psA_pool = tc.tile_pool(name="psA", bufs=6, space="PSUM")
    a2a_insts = {}
    with ln1_pool as lp, qkv_pool as qp, psA_pool as psA:
        # weight chunks, issued in consumption order on the SP queue
        aw_sb = {}
        for which, base in (("v", 2 * C), ("q", 0), ("k", C)):
            for c in range(NCT):
                tl = lp.tile([128, C], BF16, name=f"aw_{which}_{c}",
                             tag="aw", bufs=16)
                nc.sync.dma_start(out=tl,
                                  in_=w_attn[c * 128:(c + 1) * 128,
                                             base:base + C])
                aw_sb[(which, c)] = tl
        # masks stream after the attention weights (needed only when the
        # attention loop starts)
        mask0 = consts.tile([128, 4 * QCH], BF16, name="mask0")
        nc.sync.dma_start(out=mask0, in_=mask0_d[:, :])
        mask1 = consts.tile([128, 4 * QCH], BF16, name="mask1")
        nc.sync.dma_start(out=mask1, in_=mask1_d[:, :])
        mask0x = consts.tile([128, 2 * QCH], BF16, name="mask0x")
        nc.sync.dma_start(out=mask0x, in_=mask0x_d[:, :])
        mask1x = consts.tile([128, 2 * QCH], BF16, name="mask1x")
        nc.sync.dma_start(out=mask1x, in_=mask1x_d[:, :])
        eye_sb = consts.tile([128, 128], BF16, name="eye_sb")
        nc.sync.dma_start(out=eye_sb, in_=eye_d[:, :])

        def load_const(name, dram_t, shape, dtype=F32):
            t = consts.tile(shape, dtype, name=name)
            nc.sync.dma_start(out=t, in_=dram_t[:, :])
            return t

        bq_sb = load_const("bq_sb", b_q, [128, NCT]) if use_bq else None
        bk_sb = load_const("bk_sb", b_k, [128, NCT]) if use_bk else None
        bv_sb = load_const("bv_sb", b_v, [1, C], BF16) if use_bv else None
        bproj_sb = load_const("bproj_sb", b_proj, [128, NCT]) if use_bproj else None
        bfc_sb = load_const("bfc_sb", b_fc, [128, NFT]) if use_bfc else None
        bmlp_sb = load_const("bmlp_sb", b_mlp, [128, NCT]) if use_bmlp else None
        ln1w_sb = load_const("ln1w_sb", ln1w_d, [128, NCT]) if use_ln1wb else None
        ln1b_sb = load_const("ln1b_sb", ln1b_d, [128, NCT]) if use_ln1wb else None
        ln2w_sb = load_const("ln2w_sb", ln2w_d, [128, NCT]) if use_ln2wb else None
        ln2b_sb = load_const("ln2b_sb", ln2b_d, [128, NCT]) if use_ln2wb else None

        # LN1
        for c in range(NCT):
            st1 = ln_stats("ln1", lp, psA, xt_sb, c)
        ln1t = ln_finish("ln1", lp, psA, xt_sb, st1, ln1w_sb, ln1b_sb,
                         use_ln1wb)
        if debug:
            for c in range(NCT):
                nc.sync.dma_start(out=dbg["d_ln1"][c * 128:(c + 1) * 128, :],
                                  in_=ln1t[c])

        # V, token-major, assembled in one SBUF tile then scattered
        # with one DMA per (destination block, head); V rides the FIRST
        # AllToAll because the attention loop needs it only a few kt steps
        # after the first scores, while q/k are needed immediately after --
        # so v transfers while the qk AllToAll still runs
        v_all = qp.tile([128, NTT, 2 * 512], BF16, name="v_all")
        for tt in range(NTT):
            for half in range(2):
                ps = psA.tile([128, 512], F32, name=f"vps_{tt}_{half}", tag="ps")
                for c in range(NCT):
                    nc.tensor.matmul(
                        ps[:, :],
                        ln1t[c][:, tt * 128:(tt + 1) * 128],
                        aw_sb[("v", c)][:, half * 512:(half + 1) * 512],
                        start=(c == 0), stop=(c == NCT - 1 and not use_bv))
                if use_bv:
                    nc.tensor.matmul(
                        ps[:, :], ones_row[:, :],
                        bv_sb[:, half * 512:(half + 1) * 512],
                        start=False, stop=True)
                nc.vector.tensor_copy(
                    v_all[:, tt, half * 512:(half + 1) * 512], ps[:, :])
                if debug:
                    o = qp.tile([128, 512], BF16, name=f"vdbg_{tt}_{half}",
                                tag="vdbg", bufs=2)
                    nc.vector.tensor_copy(o, ps[:, :])
                    nc.sync.dma_start(
                        out=dbg["d_v"][tt * 128:(tt + 1) * 128,
                                       half * 512:(half + 1) * 512],
                        in_=o)
        for j in range(CORES):
            for h in range(2):
                nc.scalar.dma_start(
                    out=cc2_in[j, h].rearrange("(a p) f -> p a f", p=128),
                    in_=v_all[:, :, j * 128 + h * 64:j * 128 + (h + 1) * 64])

        a2a_insts["v"] = nc.gpsimd.collective_compute(
            "AllToAll", ALU.bypass,
            replica_groups=[list(range(CORES))],
            ins=[cc2_in[:, :, :].opt()],
            outs=[cc2_out[:, :, :].opt()])

        # Q^T and K^T, feature-major [C, S], into ONE merged AllToAll
        for which, bias_sb, useb, row0 in (
                ("q", bq_sb, use_bq, 0), ("k", bk_sb, use_bk, 128)):
            for hp in range(NCT):
                ps = psA.tile([128, 512], F32, name=f"{which}ps_{hp}", tag="ps")
                for c in range(NCT):
                    nc.tensor.matmul(
                        ps[:, :],
                        aw_sb[(which, c)][:, hp * 128:(hp + 1) * 128],
                        ln1t[c][:, :],
                        start=(c == 0), stop=(c == NCT - 1))
                o = qp.tile([128, S], FP8, name=f"{which}t_{hp}",
                            tag=f"{which}t", bufs=2)
                if useb:
                    nc.vector.tensor_scalar_add(o, ps[:, :],
                                                bias_sb[:, hp:hp + 1])
                else:
                    nc.vector.tensor_copy(o, ps[:, :])
                nc.scalar.dma_start(out=ccqk_in[hp, row0:row0 + 128, :],
                                    in_=o)
                if debug:
                    nm = "d_qt" if which == "q" else "d_kt"
                    nc.sync.dma_start(out=dbg[nm][hp * 128:(hp + 1) * 128, :],
                                      in_=o)
        a2a_insts["qk"] = nc.gpsimd.collective_compute(
            "AllToAll", ALU.bypass,
            replica_groups=[list(range(CORES))],
            ins=[ccqk_in[:, :, :].opt()],
            outs=[ccqk_out[:, :, :].opt()])

    # proj weights: prefetch during attention (SP queue, after masks)
    fw_dmas = {}
    pw_sb = []
    for c in range(NCT):
        tl = fw_p.tile([128, C], BF16, name=f"pw_{c}")
        d = nc.sync.dma_start(out=tl, in_=w_proj[c * 128:(c + 1) * 128, :])
        fw_dmas[("pw", c)] = d
        pw_sb.append(tl)

    # fc weights: quarter-slabs [128, 1024] with a 24-slot rotation (three
    # quarters resident). Allocated (and DMAs issued) before the attention
    # pool opens so the SBUF zone is fresh (no WAR deps on attention tiles);
    # transfers are gated onto attention-phase anchors below, streaming
    # through the attention window when DMA is otherwise idle. Quarter 3's
    # slots free as quarter 0 is consumed in the fc loop.
    fw_sb = {}
    for quarter in range(4):
        for c in range(NCT):
            tl = fw_p.tile([128, FF // 4], BF16, name=f"fw_{quarter}_{c}",
                           tag="fw", bufs=24)
            fw_dmas[(quarter, c)] = nc.sync.dma_start(
                out=tl,
                in_=w_fc[c * 128:(c + 1) * 128,
                         quarter * (FF // 4):(quarter + 1) * (FF // 4)])
            fw_sb[(quarter, c)] = tl

    # =========================================================
    # Phase 3: attention (my 2 heads, all tokens)
    # =========================================================
    att_pool = tc.tile_pool(name="att_pool", bufs=1)
    yta_pool = tc.tile_pool(name="yta_pool", bufs=1)
    psB_pool = tc.tile_pool(name="psB", bufs=2, space="PSUM")
    with att_pool as ap, psB_pool as psB:
        for b in range(B):
            nc.scalar.dma_start(
                out=qtb[b][:, :].rearrange("p (r s) -> p r s", r=4),
                in_=ccqk_out[4 * b:4 * b + 4, 0:128, :].rearrange(
                    "r p s -> p r s"))
            nc.scalar.dma_start(
                out=ktb[b][:, :].rearrange("p (r s) -> p r s", r=4),
                in_=ccqk_out[4 * b:4 * b + 4, 128:256, :].rearrange(
                    "r p s -> p r s"))
            for r4 in range(4):
                nc.gpsimd.dma_start(
                    out=vtb[(b, r4)][:, :, :, 64:128],
                    in_=cc2_out[4 * b + r4].rearrange(
                        "h (a p) f -> p h a f", p=128))

        anchors = {}
        pending = []

        def flush_normalize(item):
            fb, fp, y_A, y_B, ytAB = item
            for hh, y_ps in enumerate((y_A, y_B)):
                rec = ap.tile([1, W2], F32, name=f"rec_{fb}_{fp}_{hh}",
                              tag="rec", bufs=4)
                nc.vector.reciprocal_approx_fast(rec, y_ps[0:1, :])
                den = ap.tile([64, W2], F32, name=f"den_{fb}_{fp}_{hh}",
                              tag="den", bufs=4)
                nc.gpsimd.partition_broadcast(den, rec)
                nc.vector.tensor_mul(ytAB[hh * 64:(hh + 1) * 64, :],
                                     y_ps[64:128, :], den)
            nc.gpsimd.dma_start(out=cc3_in[4 * fb + fp], in_=ytAB)
        # process query chunks in PAIRS (qc, qc+1): shared key tiles get one
        # N=512 matmul covering both chunks' queries; the pair's last two key
        # tiles (diagonal of chunk qc+1) run N=256 on chunk qc+1 only.
        W2 = 2 * QCH
        npair = 0
        for b in range(B):
            for p in reversed(range(NQC // 2)):
                qc = 2 * p
                qs = qc * QCH
                nsh = 2 * (qc + 1)          # shared key tiles
                # y accumulators: rows 0=den, 64:128=y; cols = 2 chunks
                y_A = psB.tile([128, W2], F32, name=f"yA_{b}_{p}", tag="ya",
                               bufs=4)
                y_B = psB.tile([128, W2], F32, name=f"yB_{b}_{p}", tag="ya",
                               bufs=4)
                # normalized y (head A rows 0:64, head B rows 64:128)
                ytAB = ap.tile([128, W2], BF16, name=f"ytab_{b}_{p}",
                               tag="ytAB", bufs=4)
                flush_due = pending.pop(0) if pending else None
                for kt in range(nsh + 2):
                    if kt == 1 and flush_due is not None:
                        flush_normalize(flush_due)
                        flush_due = None
                    shared = kt < nsh
                    cols = slice(0, W2) if shared else slice(QCH, W2)
                    ncols = W2 if shared else QCH
                    # scores for both heads into one 2-bank PSUM tile
                    s_AB = psB.tile([128, 2 * W2], F32, name=f"s_{b}_{p}_{kt}",
                                    tag="ps2", bufs=2)
                    nc.tensor.matmul(s_AB[:, 0:ncols],
                                     ktb[b][0:64, kt * 128:(kt + 1) * 128],
                                     qtb[b][0:64, qs + cols.start:qs + W2],
                                     start=True, stop=True)
                    nc.tensor.matmul(s_AB[:, W2:W2 + ncols],
                                     ktb[b][64:128, kt * 128:(kt + 1) * 128],
                                     qtb[b][64:128, qs + cols.start:qs + W2],
                                     start=True, stop=True)
                    e_AB = ap.tile([128, 2 * W2], BF16, name=f"e_{b}_{p}_{kt}",
                                   tag="eAB", bufs=8)
                    if shared:
                        nc.scalar.activation(e_AB, s_AB[:, :], AF.Exp,
                                             scale=1.0 / np.sqrt(HS))
                        if kt == qc * 2:
                            nc.vector.tensor_mul(e_AB, e_AB, mask0)
                        elif kt == qc * 2 + 1:
                            nc.vector.tensor_mul(e_AB, e_AB, mask1)
                    else:
                        e3 = e_AB.rearrange("p (h q) -> p h q", h=2)
                        s3 = s_AB.rearrange("p (h q) -> p h q", h=2)
                        nc.scalar.activation(e3[:, :, 0:QCH], s3[:, :, 0:QCH],
                                             AF.Exp, scale=1.0 / np.sqrt(HS))
                        mx = mask0x if kt == nsh else mask1x
                        nc.vector.tensor_mul(
                            e_AB.rearrange("p (h q) -> p h q", h=2)[:, :, 0:QCH],
                            e_AB.rearrange("p (h q) -> p h q", h=2)[:, :, 0:QCH],
                            mx.rearrange("p (h q) -> p h q", h=2))
                    v3 = vtb[(b, kt // 4)]
                    q4 = kt % 4
                    nc.tensor.matmul(y_A[:, cols], v3[:, 0, q4, :],
                                     e_AB[:, 0:ncols],
                                     start=(kt == 0), stop=(kt == nsh + 1),
                                     skip_group_check=True)
                    mmB = nc.tensor.matmul(y_B[:, cols], v3[:, 1, q4, :],
                                           e_AB[:, W2:W2 + ncols],
                                           start=(kt == 0),
                                           stop=(kt == nsh + 1),
                                           skip_group_check=True)
                    if kt == nsh + 1:
                        anchors[npair] = mmB
                # normalize is deferred one pair (flushed inside the NEXT
                # pair's kt loop) so its vector ops interleave mid-stream
                pending.append((b, p, y_A, y_B, ytAB))
                npair += 1
        while pending:
            flush_normalize(pending.pop(0))
        a2a_y = nc.gpsimd.collective_compute(
            "AllToAll", ALU.bypass,
            replica_groups=[list(range(CORES))],
            ins=[cc3_in[:, :, :].opt()],
            outs=[cc3_out[:, :, :].opt()])

    vtq_cm.__exit__(None, None, None)  # free qtb/ktb/vt SBUF

    # now that anchors exist, gate the fc weight stream onto them
    # (quarter 3 is additionally slot-gated on quarter 0's consumption)
    for c in range(NCT):
        _delay_after(anchors[0], fw_dmas[("pw", c)])
    for quarter in range(4):
        gate = anchors[[0, 1, 2, 4][quarter]]
        for c in range(NCT):
            _delay_after(gate, fw_dmas[(quarter, c)])

    # =========================================================
    # Phase 4: proj + residual (+ interleaved LN2 stats)
    # =========================================================
    mlp_pool = tc.tile_pool(name="mlp_pool", bufs=1)
    psC_cm = tc.tile_pool(name="psC", bufs=6, space="PSUM")
    psC = psC_cm.__enter__()
    with yta_pool as yp, mlp_pool as mp:
        yta_big = yp.tile([128, NCT, S], BF16, name="yta_big")
        nc.gpsimd.dma_start(out=yta_big,
                            in_=cc3_out[:, :, :].rearrange("h p s -> p h s"))
        yta = [yta_big[:, hp, :] for hp in range(NCT)]
        if debug:
            for hp in range(NCT):
                nc.sync.dma_start(out=dbg["d_yt"][hp * 128:(hp + 1) * 128, :],
                                  in_=yta[hp])
        # keep the PE clock ramped through the y-AllToAll wait: a chain of
        # dependency-free rank-1 matmuls runs back-to-back in the gap (the
        # following proj matmuls are data-gated on yta)
        warm_ps = psC.tile([128, 512], F32, name="warm_ps", tag="ps")
        for w in range(115):
            nc.tensor.matmul(warm_ps[0:1, :], ones_col[:, :],
                             pw_sb[0][:, 0:512], start=True, stop=True,
                             skip_group_check=True)
        x2bf_sb = []
        for co in range(NCT):
            ps = psC.tile([128, 512], F32, name=f"prps_{co}", tag="ps")
            for ci in range(NCT):
                nc.tensor.matmul(ps[:, :],
                                 pw_sb[ci][:, co * 128:(co + 1) * 128],
                                 yta[ci],
                                 start=(ci == 0), stop=False)
            # residual add on the PE: accumulate eye @ x (exact in bf16)
            nc.tensor.matmul(ps[:, :], eye_sb[:, :], xt_sb[co][:, :],
                             start=False, stop=True)
            x2b = x2t_p.tile([128, S], BF16, name=f"x2bf_{co}")
            if use_bproj:
                nc.vector.tensor_scalar_add(x2b, ps[:, :],
                                            bproj_sb[:, co:co + 1])
            else:
                nc.scalar.copy(x2b, ps[:, :])
            x2bf_sb.append(x2b)
            # LN2 stats ride along so the finish chain starts immediately
            st2 = ln_stats("ln2", mp, psC, x2bf_sb, co)
            if debug:
                nc.sync.dma_start(out=dbg["d_x2"][co * 128:(co + 1) * 128, :],
                                  in_=x2b)
        xt_cm.__exit__(None, None, None)  # free xt SBUF

        # Phase 5: LN2 finish
        ln2t = ln_finish("ln2", mp, psC, x2bf_sb, st2, ln2w_sb, ln2b_sb,
                         use_ln2wb)
        if debug:
            for c in range(NCT):
                nc.sync.dma_start(out=dbg["d_ln2"][c * 128:(c + 1) * 128, :],
                                  in_=ln2t[c])

        # Phase 6: fc + GELU
        ht = []
        for f in range(NFT):
            quarter, fo = f // (NFT // 4), f % (NFT // 4)
            ps = psC.tile([128, 512], F32, name=f"fcps_{f}", tag="ps")
            for c in range(NCT):
                nc.tensor.matmul(ps[:, :],
                                 fw_sb[(quarter, c)][:, fo * 128:(fo + 1) * 128],
                                 ln2t[c][:, :],
                                 start=(c == 0), stop=(c == NCT - 1))
            h = mp.tile([128, S], BF16, name=f"ht_{f}")
            if use_bfc:
                nc.scalar.activation(h, ps[:, :], AF.Gelu,
                                     bias=bfc_sb[:, f:f + 1], scale=1.0)
            else:
                nc.scalar.activation(h, ps[:, :], AF.Gelu, scale=1.0)
            ht.append(h)

        # Phase 7: mlp proj + residual -> out (f-outer, 8 PSUM accumulators)
        psC_cm.__exit__(None, None, None)
        psM_cm = tc.tile_pool(name="psM", bufs=8, space="PSUM")
        psM = psM_cm.__enter__()
        accs = [psM.tile([128, 512], F32, name=f"mlps_{co}", tag="psm",
                         bufs=8) for co in range(NCT)]
        # f-outer for the first 24 hidden tiles (weights stream through a
        # rotating pool); the last 8 run co-outer so the output slabs finish
        # STAGGERED and their PSUM copy + store overlap the remaining matmuls
        NTAIL = 8
        mw_tail = []
        for f in range(NFT):
            tl = mp.tile([128, C], BF16, name=f"mw_{f}", tag="mw", bufs=12)
            d = nc.sync.dma_start(out=tl, in_=w_mlp[f * 128:(f + 1) * 128, :])
            _delay_after(anchors[6], d)
            if f < NFT - NTAIL:
                for co in range(NCT):
                    nc.tensor.matmul(accs[co][:, :],
                                     tl[:, co * 128:(co + 1) * 128],
                                     ht[f][:, :],
                                     start=(f == 0), stop=False)
            else:
                mw_tail.append((f, tl))
        for co in range(NCT):
            for f, tl in mw_tail:
                nc.tensor.matmul(accs[co][:, :],
                                 tl[:, co * 128:(co + 1) * 128],
                                 ht[f][:, :], start=False, stop=False)
            # residual add on the PE, then a scalar-engine copy out of PSUM
            nc.tensor.matmul(accs[co][:, :], eye_sb[:, :],
                             x2bf_sb[co][:, :], start=False, stop=True)
            o = mp.tile([128, S], BF16, name=f"out_{co}", tag="outt", bufs=3)
            if use_bmlp:
                nc.vector.tensor_scalar_add(o, accs[co][:, :],
                                            bmlp_sb[:, co:co + 1])
            else:
                nc.scalar.copy(o, accs[co][:, :])
            nc.scalar.dma_start(out=out_d[co * 128:(co + 1) * 128, :], in_=o)
        psM_cm.__exit__(None, None, None)

    es.close()


# =============================================================
# Host side
# =============================================================
_CACHE = {}


def _get_nc(flags):
    if flags not in _CACHE:
        _CACHE[flags] = build(flags)
    return _CACHE[flags]


def _prep(inputs, debug=False):
    f32 = np.float32
    x = np.asarray(inputs["x"], f32)
    attn_w = np.asarray(inputs["attn_w"], f32)
    attn_b = np.asarray(inputs["attn_b"], f32)
    proj_w = np.asarray(inputs["proj_w"], f32)
    proj_b = np.asarray(inputs["proj_b"], f32)
    fc_w = np.asarray(inputs["fc_w"], f32)
    fc_b = np.asarray(inputs["fc_b"], f32)
    mlp_w = np.asarray(inputs["mlp_proj_w"], f32)
    mlp_b = np.asarray(inputs["mlp_proj_b"], f32)
    ln1w = np.asarray(inputs["ln1_w"], f32)
    ln1b = np.asarray(inputs["ln1_b"], f32)
    ln2w = np.asarray(inputs["ln2_w"], f32)
    ln2b = np.asarray(inputs["ln2_b"], f32)

    def nz(a):
        return bool(np.any(a != 0.0))

    flags = (nz(attn_b[0:C]), nz(attn_b[C:2 * C]), nz(attn_b[2 * C:3 * C]),
             nz(proj_b), nz(fc_b), nz(mlp_b),
             nz(ln1w - 1.0) or nz(ln1b), nz(ln2w - 1.0) or nz(ln2b), debug)

    def colsplit(v):
        # [n*128] -> [128, n] with col j = v[j*128:(j+1)*128]
        return np.ascontiguousarray(v.reshape(-1, 128).T)

    bf = lambda a: np.ascontiguousarray(a).astype(NP_BF16)
    shared = {
        "w_attn": bf(attn_w), "w_proj": bf(proj_w), "w_fc": bf(fc_w),
        "w_mlp": bf(mlp_w),
        "b_q": colsplit(attn_b[0:C]), "b_k": colsplit(attn_b[C:2 * C]),
        "b_v": bf(attn_b[2 * C:3 * C].reshape(1, C)),
        "b_proj": colsplit(proj_b), "b_fc": colsplit(fc_b),
        "b_mlp": colsplit(mlp_b),
        "ln1w": colsplit(ln1w), "ln1b": colsplit(ln1b),
        "ln2w": colsplit(ln2w), "ln2b": colsplit(ln2b),
    }
    k_idx = np.arange(128)[:, None]
    q_idx = np.arange(QCH)[None, :]
    m0 = (q_idx >= k_idx).astype(NP_BF16)
    m1 = (q_idx >= k_idx + 128).astype(NP_BF16)
    one = np.ones((128, QCH), NP_BF16)
    shared["mask0"] = np.concatenate([m0, one, m0, one], axis=1)
    shared["mask1"] = np.concatenate([m1, one, m1, one], axis=1)
    shared["mask0x"] = np.concatenate([m0, m0], axis=1)
    shared["mask1x"] = np.concatenate([m1, m1], axis=1)
    shared["eye"] = np.eye(128, dtype=NP_BF16)

    in_maps = []
    for i in range(CORES):
        b, s = i // 4, i % 4
        xs = np.ascontiguousarray(x[b, s * S:(s + 1) * S, :].T)  # [C, S]
        m = dict(shared)
        m["xt"] = xs.astype(NP_BF16)
        in_maps.append(m)
    return flags, in_maps


def run_sharded(inputs, debug=False, trace=False, trace_kwargs=None):
    flags, in_maps = _prep(inputs, debug)
    nc = _get_nc(flags)
    res = bass_utils.run_bass_kernel_spmd(
        nc, in_maps, core_ids=list(range(CORES)), trace=trace,
        **(trace_kwargs or {}))
    out = np.empty((B, T, C), np.float32)
    for i in range(CORES):
        b, s = i // 4, i % 4
        out[b, s * S:(s + 1) * S, :] = np.asarray(
            res.results[i]["out"], np.float32).T
    return out, res


def kernel(**inputs):
    out, _ = run_sharded(inputs, debug=False, trace=False)
    return out
